# revision 2
# baseline (speedup 1.0000x reference)
"""Trainium2 Bass kernel for nn_BDFM_Multi (B=8,C=256,H=W=128,N=4).

Data-parallel over batch: one batch element per NeuronCore (8 cores).

v2: bf16 datapath + XBAR DMA-transpose (no PE transposes).

Per-core computation (feature f [C,HW] bf16, m [N,H,W] f32, HW=16384):
  z    = (m > 0.3)                                  binary
  er   = 13-tap separable min-filter(z), dl = 13-tap separable max-filter(z)
         -> banded 0/1 matmuls + thresholds (exact on binary data)
  fbu  = per-class channels (er, 1-dl, dl-er)       [12, HW]
  mid  = fbu @ f^T                                  [12, C]
  A'   = Wo2' @ mid^T ; G = A' @ mid ; Wc = Wo1' + G @ Wf' ; u = G beta_f + beta_o
  out  = Wc @ f + u                                 [C, HW]

Dataflow:
  - f loaded bf16 (8 MB) on SWDGE in 16 chunks
  - f^T built by XBAR DMA-transpose SBUF->SBUF (sync/scalar HWDGE), chasing
    the load; mid matmuls chase the transpose
  - small stage in fp32r; Wc cast to bf16
  - pass 2: bf16 matmuls + bias-add -> bf16 out (8.4 MB); host casts to f32
"""
import numpy as np
from contextlib import ExitStack

import ml_dtypes

import concourse.bass as bass
import concourse.mybir as mybir
import concourse.tile as tile
from concourse import bacc
from concourse import bass_utils

F32 = mybir.dt.float32
F32R = mybir.dt.float32r
BF16 = mybir.dt.bfloat16
ALU = mybir.AluOpType
ACTF = mybir.ActivationFunctionType

B, C, H, W, N = 8, 256, 128, 128, 4
HW = H * W
EPS = 1e-5
P = 128
PT = 512              # p-tile width for pass 2
LCW = 2048            # load-chunk width (columns) per block
NLC = HW // LCW       # 8 load chunks per block
GH = LCW // P         # 16 h-chunks per transpose piece

_NC_CACHE = {}


def _band_consts():
    idx = np.arange(P)
    # erosion: output i covers input [i-8, i+4]; dilation: [i-4, i+8]
    band_er = ((idx[:, None] >= idx[None, :] - 8) &
               (idx[:, None] <= idx[None, :] + 4)).astype(np.float32)
    band_dl = ((idx[:, None] >= idx[None, :] - 4) &
               (idx[:, None] <= idx[None, :] + 8)).astype(np.float32)
    cnt_er = band_er.sum(axis=0, dtype=np.float32).reshape(P, 1)
    return band_er, band_dl, cnt_er


# pk  (f32 -> fp32r cast-load) [128, 1152]: wo2t 512 | wfn 512 | ident 128
# pkb (bf16)  [128, 384]: band_er 128 | band_dl 128 | ident 128
# pkf (f32)   [128, 1029]: m 512 | wo1t 512 | betaf 2 | betao 2 | cnt 1
PK_W = 1152
PKB_W = 384
PKF_W = 1029


def build():
    if "nc" in _NC_CACHE:
        return _NC_CACHE["nc"]
    nc = bacc.Bacc(trn_type="TRN2", target_bir_lowering=False, debug=False)

    feature = nc.dram_tensor("feature", [C, HW], BF16, kind="ExternalInput")
    pk = nc.dram_tensor("pk", [P, PK_W], F32, kind="ExternalInput")
    pkb = nc.dram_tensor("pkb", [P, PKB_W], BF16, kind="ExternalInput")
    pkf = nc.dram_tensor("pkf", [P, PKF_W], F32, kind="ExternalInput")
    out = nc.dram_tensor("out", [C, HW], BF16, kind="ExternalOutput")

    with tile.TileContext(nc) as tc, ExitStack() as ctx:
        persist = ctx.enter_context(tc.tile_pool(name="persist", bufs=1))

        # ---------------- loads ----------------
        # small params first on the HWDGE queues (morphology needs them)
        pkb_sb = persist.tile([P, PKB_W], BF16)
        nc.sync.dma_start(out=pkb_sb[:], in_=pkb[:])
        bander_sb = pkb_sb[:, 0:128]
        banddl_sb = pkb_sb[:, 128:256]
        ident_bf = pkb_sb[:, 256:384]

        pkf_sb = persist.tile([P, PKF_W], F32)
        nc.scalar.dma_start(out=pkf_sb[:], in_=pkf[:])
        m_sb = pkf_sb[:, 0:512]
        wo1_sb = pkf_sb[:, 512:1024]
        betaf_sb = pkf_sb[:, 1024:1026]
        betao_sb = pkf_sb[:, 1026:1028]
        cnt_sb = pkf_sb[:, 1028:1029]

        # big feature load on the SWDGE queue, block-interleaved
        feat = persist.tile([P, 2 * HW], BF16)   # c-blk0 | c-blk1
        for q in range(NLC):
            for blk in range(2):
                nc.gpsimd.dma_start(
                    out=feat[:, blk * HW + q * LCW: blk * HW + (q + 1) * LCW],
                    in_=feature[blk * P:(blk + 1) * P, q * LCW:(q + 1) * LCW])
        # fp32r params for the small stage (cast-load must be SWDGE); needed
        # only ~when the load finishes, so queued after the feature
        pk_sb = persist.tile([P, PK_W], F32R)
        nc.gpsimd.dma_start(out=pk_sb[:], in_=pk[:])
        wo2_sb = pk_sb[:, 0:512]
        wfn_sb = pk_sb[:, 512:1024]
        ident_r = pk_sb[:, 1024:1152]

        # f^T via XBAR DMA-transpose, chasing the load chunks
        fTall = persist.tile([P, 2 * HW], BF16)  # [w, blk*HW + h*128 + c]
        fT_v = fTall.rearrange("w (b h c) -> w b h c", b=2, c=P)
        for q in range(NLC):
            for blk in range(2):
                eng = nc.sync if (q * 2 + blk) % 2 == 0 else nc.scalar
                eng.dma_start(
                    out=fT_v[:, blk, q * GH:(q + 1) * GH, :],
                    in_=feat[:, blk * HW + q * LCW: blk * HW + (q + 1) * LCW],
                    transpose=True)
        fT_mid = fTall.rearrange("w (b hc) -> w b hc", b=2)

        # persistent results
        fbuT = persist.tile([P, P * 12], BF16)     # [w, h*12 + k]
        wc_bf = persist.tile([P, 512], BF16)       # Wc^T blocks (a*2+o), bf16
        u_sb = persist.tile([P, 2], F32)           # bias per o-blk
        mid_r = persist.tile([12, 256], F32R)

        wsrc = persist.tile([P, 512], BF16)

        # ---------------- morphology (+ PE warm-up) ----------------
        with tc.tile_pool(name="morph", bufs=1) as mo, \
             tc.tile_pool(name="morph_ps", bufs=2, space="PSUM") as mops:
            # bridge the HAM cold window with zero-dependency dummy matmuls
            nc.vector.memset(wsrc[:], 0.0)
            wp = mops.tile([P, 512], F32, tag="mps")
            for i in range(8):
                nc.tensor.matmul(wp[:], wsrc[:, 0:P], wsrc[:],
                                 start=True, stop=True, skip_group_check=True)

            z_sb = mo.tile([P, N * P], BF16)
            nc.vector.tensor_scalar(z_sb[:], m_sb, 0.3, None, op0=ALU.is_gt)

            ps_rows_er = mops.tile([P, N * P], F32, tag="mps")
            nc.tensor.matmul(ps_rows_er[:], bander_sb, z_sb[:],
                             start=True, stop=True)
            ps_rows_dl = mops.tile([P, N * P], F32, tag="mps")
            nc.tensor.matmul(ps_rows_dl[:], banddl_sb, z_sb[:],
                             start=True, stop=True)

            rows_er = mo.tile([P, N * P], BF16)
            nc.vector.tensor_scalar(rows_er[:], ps_rows_er[:], cnt_sb, None,
                                    op0=ALU.is_equal)
            rows_dl = mo.tile([P, N * P], BF16)
            nc.vector.tensor_scalar(rows_dl[:], ps_rows_dl[:], 0.5, None,
                                    op0=ALU.is_gt)

            # transpose each class tile -> [w, h]
            rows_erT = mo.tile([P, N * P], BF16)
            rows_dlT = mo.tile([P, N * P], BF16)
            for n in range(N):
                ps_tr = mops.tile([P, 2 * P], BF16, tag="mps")
                nc.tensor.matmul(ps_tr[:, 0:P], rows_er[:, n * P:(n + 1) * P],
                                 ident_bf, is_transpose=True)
                nc.tensor.matmul(ps_tr[:, P:2 * P], rows_dl[:, n * P:(n + 1) * P],
                                 ident_bf, is_transpose=True)
                nc.vector.tensor_copy(rows_erT[:, n * P:(n + 1) * P], ps_tr[:, 0:P])
                nc.vector.tensor_copy(rows_dlT[:, n * P:(n + 1) * P], ps_tr[:, P:2 * P])

            ps_cols_er = mops.tile([P, N * P], F32, tag="mps")
            nc.tensor.matmul(ps_cols_er[:], bander_sb, rows_erT[:],
                             start=True, stop=True)
            ps_cols_dl = mops.tile([P, N * P], F32, tag="mps")
            nc.tensor.matmul(ps_cols_dl[:], banddl_sb, rows_dlT[:],
                             start=True, stop=True)

            er_t = mo.tile([P, N * P], BF16)   # er^T per class [w, h]
            dl_t = mo.tile([P, N * P], BF16)
            nc.vector.tensor_scalar(er_t[:], ps_cols_er[:], cnt_sb, None,
                                    op0=ALU.is_equal)
            nc.vector.tensor_scalar(dl_t[:], ps_cols_dl[:], 0.5, None, op0=ALU.is_gt)

            # write channels into fbuT at [w, h*12 + k], k = 3n+j
            fbuT_v = fbuT.rearrange("w (h k) -> w h k", k=12)
            for n in range(N):
                src_er = er_t[:, n * P:(n + 1) * P]
                src_dl = dl_t[:, n * P:(n + 1) * P]
                nc.vector.tensor_copy(fbuT_v[:, :, 3 * n], src_er)
                nc.vector.tensor_scalar(fbuT_v[:, :, 3 * n + 1], src_dl, 0.0, None,
                                        op0=ALU.is_equal)
                nc.vector.tensor_tensor(fbuT_v[:, :, 3 * n + 2], src_dl, src_er,
                                        op=ALU.subtract)

        # ---------------- pass 1: mid = fbu @ f^T ----------------
        with tc.tile_pool(name="mid_ps", bufs=1, space="PSUM") as midps:
            mid_ps = midps.tile([12, 256], F32)
            for h in range(P):
                nc.tensor.matmul(mid_ps[:], fbuT[:, h * 12:h * 12 + 12],
                                 fT_mid[:, :, h * P:(h + 1) * P],
                                 start=(h == 0), stop=(h == P - 1),
                                 skip_group_check=True)
            nc.vector.tensor_copy(mid_r[:], mid_ps[:])

        # ---------------- small stage: mid^T, A'^T, G^T, Wc, u ----------------
        with tc.tile_pool(name="sm_ps", bufs=1, space="PSUM") as smps, \
             tc.tile_pool(name="sm_sb", bufs=1) as smsb:
            # mid^T via PE transpose of [12,128] chunks (fp32r)
            ps_mt = smps.tile([P, 24], F32R, tag="mt")
            for ci in range(2):
                nc.tensor.matmul(ps_mt[:, ci * 12:(ci + 1) * 12],
                                 mid_r[:, ci * P:(ci + 1) * P],
                                 pk_sb[0:12, 1024:1036], is_transpose=True)
            mid_t = smsb.tile([P, 24], F32R)
            nc.vector.tensor_copy(mid_t[:], ps_mt[:])

            # A'^T = mid @ Wo2'^T   [12, 256]
            ps_at = smps.tile([12, 256], F32, tag="at")
            nc.tensor.matmul(ps_at[:], mid_t[:, 0:12], wo2_sb[:, 0:256],
                             start=True, stop=False)
            nc.tensor.matmul(ps_at[:], mid_t[:, 12:24], wo2_sb[:, 256:512],
                             start=False, stop=True)
            a_t = smsb.tile([12, 256], F32R)
            nc.vector.tensor_copy(a_t[:], ps_at[:])

            # G^T[c, o] = sum_k mid[k, c] A'^T[k, o];  chunks ci on partitions
            ps_gt = smps.tile([P, 512], F32, tag="gt")
            for ci in range(2):
                nc.tensor.matmul(ps_gt[:, ci * 256:(ci + 1) * 256],
                                 mid_r[:, ci * P:(ci + 1) * P], a_t[:],
                                 start=True, stop=True)
            gt_r = smsb.tile([P, 512], F32R)
            nc.vector.tensor_copy(gt_r[:], ps_gt[:])
            gt_f = smsb.tile([P, 512], F32)
            nc.vector.tensor_copy(gt_f[:], ps_gt[:])

            # X = Wf'^T @ G^T (= (G Wf')^T); blocks a (c_in chunk) on partitions
            ps_x = smps.tile([P, 512], F32, tag="x")
            for a in range(2):
                for ci in range(2):
                    nc.tensor.matmul(ps_x[:, a * 256:(a + 1) * 256],
                                     wfn_sb[:, (ci * 2 + a) * P:(ci * 2 + a + 1) * P],
                                     gt_r[:, ci * 256:(ci + 1) * 256],
                                     start=(ci == 0), stop=(ci == 1),
                                     skip_group_check=True)
            # Wc^T = Wo1'^T + X  (blocks (a*2+o) align with [a*256 + o*128])
            for a in range(2):
                nc.vector.tensor_tensor(wc_bf[:, a * 256:(a + 1) * 256],
                                        ps_x[:, a * 256:(a + 1) * 256],
                                        wo1_sb[:, a * 256:(a + 1) * 256],
                                        op=ALU.add)

            # u = G @ beta_f + beta_o   per o-blk  (fp32 matmuls)
            for o in range(2):
                ps_u = smps.tile([P, 1], F32, tag="u")
                nc.tensor.matmul(ps_u[:], gt_f[:, o * P:(o + 1) * P],
                                 betaf_sb[:, 0:1], start=True, stop=False)
                nc.tensor.matmul(ps_u[:], gt_f[:, 256 + o * P:256 + (o + 1) * P],
                                 betaf_sb[:, 1:2], start=False, stop=True)
                nc.scalar.activation(u_sb[:, o:o + 1], ps_u[:], ACTF.Identity,
                                     bias=betao_sb[:, o:o + 1])

        # ---------------- pass 2: out = Wc @ f + u ----------------
        with tc.tile_pool(name="out_ps", bufs=4, space="PSUM") as outps, \
             tc.tile_pool(name="p2_sb", bufs=2) as p2sb:
            for tg in range(8):
                ot0 = p2sb.tile([P, 4 * PT], BF16, tag="ot0")
                ot1 = p2sb.tile([P, 4 * PT], BF16, tag="ot1")
                for tt in range(4):
                    t = tg * 4 + tt
                    c0 = t * PT
                    out_ps = outps.tile([P, 2 * PT], F32, tag="ops")
                    for o in range(2):
                        ops = out_ps[:, o * PT:(o + 1) * PT]
                        nc.tensor.matmul(ops,
                                         wc_bf[:, (0 * 2 + o) * P:(0 * 2 + o + 1) * P],
                                         feat[:, c0:c0 + PT],
                                         start=True, stop=False, skip_group_check=True)
                        nc.tensor.matmul(ops,
                                         wc_bf[:, (1 * 2 + o) * P:(1 * 2 + o + 1) * P],
                                         feat[:, HW + c0:HW + c0 + PT],
                                         start=False, stop=True, skip_group_check=True)
                    nc.scalar.activation(ot0[:, tt * PT:(tt + 1) * PT],
                                         out_ps[:, 0:PT],
                                         ACTF.Identity, bias=u_sb[:, 0:1])
                    nc.vector.tensor_scalar(ot1[:, tt * PT:(tt + 1) * PT],
                                            out_ps[:, PT:2 * PT],
                                            u_sb[:, 1:2], None, op0=ALU.add)
                g0 = tg * 4 * PT
                nc.sync.dma_start(out=out[0:P, g0:g0 + 4 * PT], in_=ot0[:])
                nc.gpsimd.dma_start(out=out[P:C, g0:g0 + 4 * PT], in_=ot1[:])

    nc.compile()
    _NC_CACHE["nc"] = nc
    return nc


def prepare_in_maps(feature, m, W_f, g_f, b_f, mu_f, v_f, W_o, g_o, b_o, mu_o, v_o):
    feature = np.asarray(feature, dtype=np.float32)
    m = np.asarray(m, dtype=np.float32)
    W_f = np.asarray(W_f, dtype=np.float32)
    W_o = np.asarray(W_o, dtype=np.float32)
    g_f, b_f, mu_f, v_f = (np.asarray(x, dtype=np.float32) for x in (g_f, b_f, mu_f, v_f))
    g_o, b_o, mu_o, v_o = (np.asarray(x, dtype=np.float32) for x in (g_o, b_o, mu_o, v_o))

    inv_f = g_f / np.sqrt(v_f + EPS)
    beta_f_v = b_f - mu_f * inv_f
    inv_o = g_o / np.sqrt(v_o + EPS)
    beta_o_v = b_o - mu_o * inv_o
    Wf_p = (inv_f[:, None] * W_f).astype(np.float32)          # [C, C]
    Wo1_p = (inv_o[:, None] * W_o[:, :C]).astype(np.float32)  # [C, C]
    Wo2_p = (inv_o[:, None] * W_o[:, C:]).astype(np.float32)  # [C, C]

    def blocks_t(Wp):
        # lhsT layout: blocks ci*2+o of Wp^T
        a = np.empty((P, 512), np.float32)
        for ci in range(2):
            for o in range(2):
                a[:, (ci * 2 + o) * P:(ci * 2 + o + 1) * P] = \
                    Wp[o * P:(o + 1) * P, ci * P:(ci + 1) * P].T
        return a

    def blocks_n(Wp):
        # natural-layout blocks ci*2+a: Wp[ci*128:(ci+1)*128, a*128:(a+1)*128]
        a_ = np.empty((P, 512), np.float32)
        for ci in range(2):
            for a in range(2):
                a_[:, (ci * 2 + a) * P:(ci * 2 + a + 1) * P] = \
                    Wp[ci * P:(ci + 1) * P, a * P:(a + 1) * P]
        return a_

    band_er, band_dl, cnt_er = _band_consts()
    pk = np.empty((P, PK_W), np.float32)
    pk[:, 0:512] = np.concatenate([Wo2_p.T[0:P, :], Wo2_p.T[P:C, :]], axis=1)
    pk[:, 512:1024] = blocks_n(Wf_p)
    pk[:, 1024:1152] = np.eye(P, dtype=np.float32)

    pkb = np.empty((P, PKB_W), np.float32)
    pkb[:, 0:128] = band_er
    pkb[:, 128:256] = band_dl
    pkb[:, 256:384] = np.eye(P, dtype=np.float32)
    pkb = pkb.astype(ml_dtypes.bfloat16)

    pkf = np.empty((P, PKF_W), np.float32)
    pkf[:, 512:1024] = blocks_t(Wo1_p)
    pkf[:, 1024:1026] = beta_f_v.reshape(2, P).T
    pkf[:, 1026:1028] = beta_o_v.reshape(2, P).T
    pkf[:, 1028:1029] = cnt_er

    in_maps = []
    for b in range(B):
        im = {"pk": pk, "pkb": pkb}
        pkf_b = pkf.copy()
        # m per class into columns [n*128:(n+1)*128]
        pkf_b[:, 0:512] = np.transpose(m[b], (1, 0, 2)).reshape(P, 512)
        im["pkf"] = pkf_b
        im["feature"] = np.ascontiguousarray(
            feature[b].reshape(C, HW)).astype(ml_dtypes.bfloat16)
        in_maps.append(im)
    return in_maps


def kernel(feature, m, W_f, g_f, b_f, mu_f, v_f, W_o, g_o, b_o, mu_o, v_o):
    nc = build()
    in_maps = prepare_in_maps(feature, m, W_f, g_f, b_f, mu_f, v_f,
                              W_o, g_o, b_o, mu_o, v_o)
    res = bass_utils.run_bass_kernel_spmd(nc, in_maps, list(range(B)))
    out = np.empty((B, C, H, W), np.float32)
    for b in range(B):
        out[b] = np.asarray(res.results[b]["out"]).astype(np.float32).reshape(C, H, W)
    return out


# revision 5
# speedup vs baseline: 1.9236x; 1.9236x over previous
"""Trainium2 Bass kernel for nn_BDFM_Multi (B=8,C=256,H=W=128,N=4).

Data-parallel over batch: one batch element per NeuronCore (8 cores).

v3: bf16 datapath (halves HBM traffic), PE transposes for f^T, 4-way
col-tiled packing for the mid matmuls.

Per-core computation (feature f [C,HW] bf16, m [N,H,W] f32, HW=16384):
  z    = (m > 0.3)                                  binary
  er   = 13-tap separable min-filter(z), dl = 13-tap separable max-filter(z)
         -> banded 0/1 matmuls + thresholds (exact on binary data)
  fbu  = per-class channels (er, 1-dl, dl-er)       [12, HW]
  mid  = fbu @ f^T                                  [12, C]
  A'   = Wo2' @ mid^T ; G = A' @ mid ; Wc = Wo1' + G @ Wf' ; u = G beta_f + beta_o
  out  = Wc @ f + u                                 [C, HW]

Dataflow:
  - f loaded bf16 (8 MB) on SWDGE in 16 chunks
  - pass 1 chases the load: PE-transpose h-chunks, packed mid matmuls
    (4 concurrent col-groups, M=12 each)
  - small stage in fp32r; Wc cast to bf16
  - pass 2: bf16 matmuls + bias-add -> bf16 out (8.4 MB); host casts to f32
"""
import numpy as np
from contextlib import ExitStack

import ml_dtypes

import concourse.bass as bass
import concourse.mybir as mybir
import concourse.tile as tile
from concourse import bacc
from concourse import bass_utils

F32 = mybir.dt.float32
F32R = mybir.dt.float32r
BF16 = mybir.dt.bfloat16
ALU = mybir.AluOpType
ACTF = mybir.ActivationFunctionType

B, C, H, W, N = 8, 256, 128, 128, 4
HW = H * W
EPS = 1e-5
P = 128
PT = 512              # p-tile width for pass 2
LCW = 2048            # load-chunk width (columns) per block
NLC = HW // LCW       # 8 load chunks per block
G1 = 4                # h-chunks per transpose group in pass 1

_NC_CACHE = {}


def _band_consts():
    idx = np.arange(P)
    # erosion: output i covers input [i-8, i+4]; dilation: [i-4, i+8]
    band_er = ((idx[:, None] >= idx[None, :] - 8) &
               (idx[:, None] <= idx[None, :] + 4)).astype(np.float32)
    band_dl = ((idx[:, None] >= idx[None, :] - 4) &
               (idx[:, None] <= idx[None, :] + 8)).astype(np.float32)
    cnt_er = band_er.sum(axis=0, dtype=np.float32).reshape(P, 1)
    return band_er, band_dl, cnt_er


# pk  (f32 -> fp32r cast-load) [128, 1152]: wo2t 512 | wfn 512 | ident 128
# pkb (bf16)  [128, 384]: band_er 128 | band_dl 128 | ident 128
# pkf (f32)   [128, 1029]: m 512 | wo1t 512 | betaf 2 | betao 2 | cnt 1
PK_W = 1152
PKB_W = 384
PKF_W = 1029


def build():
    if "nc" in _NC_CACHE:
        return _NC_CACHE["nc"]
    nc = bacc.Bacc(trn_type="TRN2", target_bir_lowering=False, debug=False)

    feature = nc.dram_tensor("feature", [C, HW], BF16, kind="ExternalInput")
    pk = nc.dram_tensor("pk", [P, PK_W], F32, kind="ExternalInput")
    pkb = nc.dram_tensor("pkb", [P, PKB_W], BF16, kind="ExternalInput")
    pkf = nc.dram_tensor("pkf", [P, PKF_W], F32, kind="ExternalInput")
    out = nc.dram_tensor("out", [C, HW], BF16, kind="ExternalOutput")

    with tile.TileContext(nc) as tc, ExitStack() as ctx:
        persist = ctx.enter_context(tc.tile_pool(name="persist", bufs=1))

        # ---------------- loads ----------------
        # small params first on the HWDGE queues (morphology needs them)
        pkb_sb = persist.tile([P, PKB_W], BF16)
        nc.sync.dma_start(out=pkb_sb[:], in_=pkb[:])
        bander_sb = pkb_sb[:, 0:128]
        banddl_sb = pkb_sb[:, 128:256]
        ident_bf = pkb_sb[:, 256:384]

        pkf_sb = persist.tile([P, PKF_W], F32)
        nc.scalar.dma_start(out=pkf_sb[:], in_=pkf[:])
        m_sb = pkf_sb[:, 0:512]
        wo1_sb = pkf_sb[:, 512:1024]
        betaf_sb = pkf_sb[:, 1024:1026]
        betao_sb = pkf_sb[:, 1026:1028]
        cnt_sb = pkf_sb[:, 1028:1029]

        # big feature load on the SWDGE queue, block-interleaved
        feat = persist.tile([P, 2 * HW], BF16)   # c-blk0 | c-blk1
        for q in range(NLC):
            for blk in range(2):
                nc.gpsimd.dma_start(
                    out=feat[:, blk * HW + q * LCW: blk * HW + (q + 1) * LCW],
                    in_=feature[blk * P:(blk + 1) * P, q * LCW:(q + 1) * LCW])
        # fp32r params for the small stage (cast-load must be SWDGE); needed
        # only ~when the load finishes, so queued after the feature
        pk_sb = persist.tile([P, PK_W], F32R)
        nc.gpsimd.dma_start(out=pk_sb[:], in_=pk[:])
        wo2_sb = pk_sb[:, 0:512]
        wfn_sb = pk_sb[:, 512:1024]

        # persistent results
        fbuT = persist.tile([P, P * 12], BF16)     # [w, h*12 + k]
        wc_bf = persist.tile([P, 512], BF16)       # Wc^T blocks (a*2+o), bf16
        u_sb = persist.tile([P, 2], F32)           # bias per o-blk
        mid_r = persist.tile([12, 256], F32R)

        wsrc = persist.tile([P, 512], BF16)

        # ---------------- morphology (+ PE warm-up) ----------------
        with tc.tile_pool(name="morph", bufs=1) as mo, \
             tc.tile_pool(name="morph_ps", bufs=2, space="PSUM") as mops:
            # bridge the HAM cold window with zero-dependency dummy matmuls
            nc.vector.memset(wsrc[:], 0.0)
            wp = mops.tile([P, 512], F32, tag="mps")
            for i in range(8):
                nc.tensor.matmul(wp[:], wsrc[:, 0:P], wsrc[:],
                                 start=True, stop=True, skip_group_check=True)

            z_sb = mo.tile([P, N * P], BF16)
            nc.vector.tensor_scalar(z_sb[:], m_sb, 0.3, None, op0=ALU.is_gt)

            ps_rows_er = mops.tile([P, N * P], F32, tag="mps")
            nc.tensor.matmul(ps_rows_er[:], bander_sb, z_sb[:],
                             start=True, stop=True)
            ps_rows_dl = mops.tile([P, N * P], F32, tag="mps")
            nc.tensor.matmul(ps_rows_dl[:], banddl_sb, z_sb[:],
                             start=True, stop=True)

            rows_er = mo.tile([P, N * P], BF16)
            nc.vector.tensor_scalar(rows_er[:], ps_rows_er[:], cnt_sb, None,
                                    op0=ALU.is_equal)
            rows_dl = mo.tile([P, N * P], BF16)
            nc.vector.tensor_scalar(rows_dl[:], ps_rows_dl[:], 0.5, None,
                                    op0=ALU.is_gt)

            # transpose each class tile -> [w, h]
            rows_erT = mo.tile([P, N * P], BF16)
            rows_dlT = mo.tile([P, N * P], BF16)
            for n in range(N):
                ps_tr = mops.tile([P, 2 * P], BF16, tag="mps")
                nc.tensor.matmul(ps_tr[:, 0:P], rows_er[:, n * P:(n + 1) * P],
                                 ident_bf, is_transpose=True)
                nc.tensor.matmul(ps_tr[:, P:2 * P], rows_dl[:, n * P:(n + 1) * P],
                                 ident_bf, is_transpose=True)
                nc.vector.tensor_copy(rows_erT[:, n * P:(n + 1) * P], ps_tr[:, 0:P])
                nc.vector.tensor_copy(rows_dlT[:, n * P:(n + 1) * P], ps_tr[:, P:2 * P])

            ps_cols_er = mops.tile([P, N * P], F32, tag="mps")
            nc.tensor.matmul(ps_cols_er[:], bander_sb, rows_erT[:],
                             start=True, stop=True)
            ps_cols_dl = mops.tile([P, N * P], F32, tag="mps")
            nc.tensor.matmul(ps_cols_dl[:], banddl_sb, rows_dlT[:],
                             start=True, stop=True)

            er_t = mo.tile([P, N * P], BF16)   # er^T per class [w, h]
            dl_t = mo.tile([P, N * P], BF16)
            nc.vector.tensor_scalar(er_t[:], ps_cols_er[:], cnt_sb, None,
                                    op0=ALU.is_equal)
            nc.vector.tensor_scalar(dl_t[:], ps_cols_dl[:], 0.5, None, op0=ALU.is_gt)

            # write channels into fbuT at [w, h*12 + k], k = 3n+j
            fbuT_v = fbuT.rearrange("w (h k) -> w h k", k=12)
            for n in range(N):
                src_er = er_t[:, n * P:(n + 1) * P]
                src_dl = dl_t[:, n * P:(n + 1) * P]
                nc.vector.tensor_copy(fbuT_v[:, :, 3 * n], src_er)
                nc.vector.tensor_scalar(fbuT_v[:, :, 3 * n + 1], src_dl, 0.0, None,
                                        op0=ALU.is_equal)
                nc.vector.tensor_tensor(fbuT_v[:, :, 3 * n + 2], src_dl, src_er,
                                        op=ALU.subtract)

        # ---------------- pass 1: feature transpose + packed mid ----------------
        # mid partials land on col-groups g=h%4 at partitions 32g..32g+11
        with tc.tile_pool(name="mid_ps", bufs=1, space="PSUM") as midps, \
             tc.tile_pool(name="p1_ps", bufs=3, space="PSUM") as p1ps, \
             tc.tile_pool(name="p1_sb", bufs=4) as p1sb:
            mid_ps = midps.tile([P, 256], F32)
            for g in range(P // G1):
                tr = p1ps.tile([P, G1 * 256], BF16, tag="tr")
                for j in range(G1):
                    h = g * G1 + j
                    nc.tensor.matmul(tr[:, j * 256:j * 256 + P],
                                     feat[:, h * P:(h + 1) * P],
                                     ident_bf, is_transpose=True)
                    nc.tensor.matmul(tr[:, j * 256 + P:(j + 1) * 256],
                                     feat[:, HW + h * P:HW + (h + 1) * P],
                                     ident_bf, is_transpose=True)
                ft = p1sb.tile([P, G1 * 256], BF16, tag="ft")
                if g % 2 == 0:
                    nc.vector.tensor_copy(ft[:], tr[:])
                else:
                    nc.scalar.copy(ft[:], tr[:])
                for j in range(G1):
                    h = g * G1 + j
                    cg = 32 * j
                    nc.tensor.matmul(mid_ps[cg:cg + 12, :],
                                     fbuT[:, h * 12:h * 12 + 12],
                                     ft[:, j * 256:(j + 1) * 256],
                                     start=(g == 0), stop=(g == P // G1 - 1),
                                     skip_group_check=True,
                                     tile_position=(0, cg))
            # mid = sum of the 4 col-group partials (one PSUM operand per op)
            with tc.tile_pool(name="mid_sb", bufs=1) as midsb:
                c0 = midsb.tile([12, 256], F32)
                c2 = midsb.tile([12, 256], F32)
                s01 = midsb.tile([12, 256], F32)
                s23 = midsb.tile([12, 256], F32)
                nc.vector.tensor_copy(c0[:], mid_ps[0:12, :])
                nc.scalar.copy(c2[:], mid_ps[64:76, :])
                nc.vector.tensor_tensor(s01[:], mid_ps[32:44, :], c0[:],
                                        op=ALU.add)
                nc.vector.tensor_tensor(s23[:], mid_ps[96:108, :], c2[:],
                                        op=ALU.add)
                nc.vector.tensor_tensor(mid_r[:], s01[:], s23[:], op=ALU.add)

        # ---------------- small stage: mid^T, A'^T, G^T, Wc, u ----------------
        with tc.tile_pool(name="sm_ps", bufs=1, space="PSUM") as smps, \
             tc.tile_pool(name="sm_sb", bufs=1) as smsb:
            # mid^T via PE transpose of [12,128] chunks (fp32r)
            ps_mt = smps.tile([P, 24], F32R, tag="mt")
            for ci in range(2):
                nc.tensor.matmul(ps_mt[:, ci * 12:(ci + 1) * 12],
                                 mid_r[:, ci * P:(ci + 1) * P],
                                 pk_sb[0:12, 1024:1036], is_transpose=True)
            mid_t = smsb.tile([P, 24], F32R)
            nc.vector.tensor_copy(mid_t[:], ps_mt[:])

            # A'^T = mid @ Wo2'^T   [12, 256]
            ps_at = smps.tile([12, 256], F32, tag="at")
            nc.tensor.matmul(ps_at[:], mid_t[:, 0:12], wo2_sb[:, 0:256],
                             start=True, stop=False)
            nc.tensor.matmul(ps_at[:], mid_t[:, 12:24], wo2_sb[:, 256:512],
                             start=False, stop=True)
            a_t = smsb.tile([12, 256], F32R)
            nc.vector.tensor_copy(a_t[:], ps_at[:])

            # G^T[c, o] = sum_k mid[k, c] A'^T[k, o];  chunks ci on partitions
            ps_gt = smps.tile([P, 512], F32, tag="gt")
            for ci in range(2):
                nc.tensor.matmul(ps_gt[:, ci * 256:(ci + 1) * 256],
                                 mid_r[:, ci * P:(ci + 1) * P], a_t[:],
                                 start=True, stop=True)
            gt_r = smsb.tile([P, 512], F32R)
            nc.vector.tensor_copy(gt_r[:], ps_gt[:])
            gt_f = smsb.tile([P, 512], F32)
            nc.vector.tensor_copy(gt_f[:], ps_gt[:])

            # X = Wf'^T @ G^T (= (G Wf')^T); blocks a (c_in chunk) on partitions
            ps_x = smps.tile([P, 512], F32, tag="x")
            for a in range(2):
                for ci in range(2):
                    nc.tensor.matmul(ps_x[:, a * 256:(a + 1) * 256],
                                     wfn_sb[:, (ci * 2 + a) * P:(ci * 2 + a + 1) * P],
                                     gt_r[:, ci * 256:(ci + 1) * 256],
                                     start=(ci == 0), stop=(ci == 1),
                                     skip_group_check=True)
            # Wc^T = Wo1'^T + X  (blocks (a*2+o) align with [a*256 + o*128])
            for a in range(2):
                nc.vector.tensor_tensor(wc_bf[:, a * 256:(a + 1) * 256],
                                        ps_x[:, a * 256:(a + 1) * 256],
                                        wo1_sb[:, a * 256:(a + 1) * 256],
                                        op=ALU.add)

            # u = G @ beta_f + beta_o   per o-blk  (fp32 matmuls)
            for o in range(2):
                ps_u = smps.tile([P, 1], F32, tag="u")
                nc.tensor.matmul(ps_u[:], gt_f[:, o * P:(o + 1) * P],
                                 betaf_sb[:, 0:1], start=True, stop=False)
                nc.tensor.matmul(ps_u[:], gt_f[:, 256 + o * P:256 + (o + 1) * P],
                                 betaf_sb[:, 1:2], start=False, stop=True)
                nc.scalar.activation(u_sb[:, o:o + 1], ps_u[:], ACTF.Identity,
                                     bias=betao_sb[:, o:o + 1])

        # ---------------- pass 2: out = Wc @ f + u ----------------
        with tc.tile_pool(name="out_ps", bufs=4, space="PSUM") as outps, \
             tc.tile_pool(name="p2_sb", bufs=2) as p2sb:
            for tg in range(8):
                ot0 = p2sb.tile([P, 4 * PT], BF16, tag="ot0")
                ot1 = p2sb.tile([P, 4 * PT], BF16, tag="ot1")
                for tt in range(4):
                    t = tg * 4 + tt
                    c0 = t * PT
                    out_ps = outps.tile([P, 2 * PT], F32, tag="ops")
                    for o in range(2):
                        ops = out_ps[:, o * PT:(o + 1) * PT]
                        nc.tensor.matmul(ops,
                                         wc_bf[:, (0 * 2 + o) * P:(0 * 2 + o + 1) * P],
                                         feat[:, c0:c0 + PT],
                                         start=True, stop=False, skip_group_check=True)
                        nc.tensor.matmul(ops,
                                         wc_bf[:, (1 * 2 + o) * P:(1 * 2 + o + 1) * P],
                                         feat[:, HW + c0:HW + c0 + PT],
                                         start=False, stop=True, skip_group_check=True)
                    nc.scalar.activation(ot0[:, tt * PT:(tt + 1) * PT],
                                         out_ps[:, 0:PT],
                                         ACTF.Identity, bias=u_sb[:, 0:1])
                    nc.vector.tensor_scalar(ot1[:, tt * PT:(tt + 1) * PT],
                                            out_ps[:, PT:2 * PT],
                                            u_sb[:, 1:2], None, op0=ALU.add)
                g0 = tg * 4 * PT
                nc.sync.dma_start(out=out[0:P, g0:g0 + 4 * PT], in_=ot0[:])
                nc.gpsimd.dma_start(out=out[P:C, g0:g0 + 4 * PT], in_=ot1[:])

    nc.compile()
    _NC_CACHE["nc"] = nc
    return nc


def prepare_in_maps(feature, m, W_f, g_f, b_f, mu_f, v_f, W_o, g_o, b_o, mu_o, v_o):
    feature = np.asarray(feature, dtype=np.float32)
    m = np.asarray(m, dtype=np.float32)
    W_f = np.asarray(W_f, dtype=np.float32)
    W_o = np.asarray(W_o, dtype=np.float32)
    g_f, b_f, mu_f, v_f = (np.asarray(x, dtype=np.float32) for x in (g_f, b_f, mu_f, v_f))
    g_o, b_o, mu_o, v_o = (np.asarray(x, dtype=np.float32) for x in (g_o, b_o, mu_o, v_o))

    inv_f = g_f / np.sqrt(v_f + EPS)
    beta_f_v = b_f - mu_f * inv_f
    inv_o = g_o / np.sqrt(v_o + EPS)
    beta_o_v = b_o - mu_o * inv_o
    Wf_p = (inv_f[:, None] * W_f).astype(np.float32)          # [C, C]
    Wo1_p = (inv_o[:, None] * W_o[:, :C]).astype(np.float32)  # [C, C]
    Wo2_p = (inv_o[:, None] * W_o[:, C:]).astype(np.float32)  # [C, C]

    def blocks_t(Wp):
        # lhsT layout: blocks ci*2+o of Wp^T
        a = np.empty((P, 512), np.float32)
        for ci in range(2):
            for o in range(2):
                a[:, (ci * 2 + o) * P:(ci * 2 + o + 1) * P] = \
                    Wp[o * P:(o + 1) * P, ci * P:(ci + 1) * P].T
        return a

    def blocks_n(Wp):
        # natural-layout blocks ci*2+a: Wp[ci*128:(ci+1)*128, a*128:(a+1)*128]
        a_ = np.empty((P, 512), np.float32)
        for ci in range(2):
            for a in range(2):
                a_[:, (ci * 2 + a) * P:(ci * 2 + a + 1) * P] = \
                    Wp[ci * P:(ci + 1) * P, a * P:(a + 1) * P]
        return a_

    band_er, band_dl, cnt_er = _band_consts()
    pk = np.empty((P, PK_W), np.float32)
    pk[:, 0:512] = np.concatenate([Wo2_p.T[0:P, :], Wo2_p.T[P:C, :]], axis=1)
    pk[:, 512:1024] = blocks_n(Wf_p)
    pk[:, 1024:1152] = np.eye(P, dtype=np.float32)

    pkb = np.empty((P, PKB_W), np.float32)
    pkb[:, 0:128] = band_er
    pkb[:, 128:256] = band_dl
    pkb[:, 256:384] = np.eye(P, dtype=np.float32)
    pkb = pkb.astype(ml_dtypes.bfloat16)

    pkf = np.empty((P, PKF_W), np.float32)
    pkf[:, 512:1024] = blocks_t(Wo1_p)
    pkf[:, 1024:1026] = beta_f_v.reshape(2, P).T
    pkf[:, 1026:1028] = beta_o_v.reshape(2, P).T
    pkf[:, 1028:1029] = cnt_er

    in_maps = []
    for b in range(B):
        im = {"pk": pk, "pkb": pkb}
        pkf_b = pkf.copy()
        # m per class into columns [n*128:(n+1)*128]
        pkf_b[:, 0:512] = np.transpose(m[b], (1, 0, 2)).reshape(P, 512)
        im["pkf"] = pkf_b
        im["feature"] = np.ascontiguousarray(
            feature[b].reshape(C, HW)).astype(ml_dtypes.bfloat16)
        in_maps.append(im)
    return in_maps


def kernel(feature, m, W_f, g_f, b_f, mu_f, v_f, W_o, g_o, b_o, mu_o, v_o):
    nc = build()
    in_maps = prepare_in_maps(feature, m, W_f, g_f, b_f, mu_f, v_f,
                              W_o, g_o, b_o, mu_o, v_o)
    res = bass_utils.run_bass_kernel_spmd(nc, in_maps, list(range(B)))
    out = np.empty((B, C, H, W), np.float32)
    for b in range(B):
        out[b] = np.asarray(res.results[b]["out"]).astype(np.float32).reshape(C, H, W)
    return out
